# revision 12
# baseline (speedup 1.0000x reference)
"""Trainium2 Bass kernel for nn_AttentionMeta_58196806861321 — v2.

Math (B=1, S=512, D=256):
    k = key + key@Wk + bk ;  q = query + query@Wq + bq ;  v = value + value@Wva + bva
    raw[sk,sq,:]  = k[sk,:] * q[sq,:]
    x             = raw + raw@Wl + bl                  (logits, [Sk,Sq,D])
    xexp          = x * exp(x - max_sq(x))             (swishmax over the QUERY axis)
    scale         = xexp / (sum_sq|xexp| + 1)
    vsum[sq,:]    = sum_sk v[sk,:] * scale[sk,sq,:]
    out           = vsum + vsum@Wvo + bvo

Design (per core, Sk sharded 8 x 64 per the key-axis hint):
  * HOST precomputes the residual linears (kk/qq/vv) and the per-key
    rescaled logits weights wmod_all[key] = diag(kk_key) @ (I+Wl) in fp16;
    wmod streams in per-key via DMA (128KB each), hidden under compute.
    The final (I+Wvo)+bvo linear (0.1% of FLOPs) and the cross-core
    partial-sum are applied on the host after gathering per-core partials.
  * PE per key: 2 m-chunks x 2 K=128 matmuls -> x_psum = logits^T
    [dout(2x128p), sq=512]; plus the previous group's diag(coeff) matmuls
    (1 key per iteration, so the PE queue never stalls on fresh coeffs).
  * ACT per key: e' = Exp(x_psum - C) (bl cancels out of the coeff
    algebra) and xb = Identity(x_psum + bl) per m-chunk, PSUM->SBUF bf16.
  * DVE (governor): xexp = xb*e' (2x, full-group batch); ssum = |xexp|
    via uint16 sign-mask (4x) + 3-level f16 pairwise-add tree (2x) +
    short fp32 reduce; maxe via 2-level TT-max tree + batched reduce;
    reciprocal/coeff col math; diag(coeff) via one broadcast TT per group.
  * vsum accumulates in PSUM over 32-key halves; each half drains as raw
    bf16 [dout, sq] straight to DRAM (no on-device collective - the
    8x2 partials are summed on the host, which is ~1ms of numpy).
  """

import os
import sys

import numpy as np

for _p in ("/opt/trn_rl_repo", "/root/.axon_site/_ro/trn_rl_repo"):
    if os.path.isdir(_p) and _p not in sys.path:
        sys.path.append(_p)

import ml_dtypes  # noqa: E402

import concourse.bacc as bacc  # noqa: E402
import concourse.tile as tile  # noqa: E402
from concourse import mybir  # noqa: E402
from concourse.bass_utils import run_bass_kernel_spmd  # noqa: E402

F32 = mybir.dt.float32
F16 = mybir.dt.float16
BF16 = mybir.dt.bfloat16
AX = mybir.AxisListType
ALU = mybir.AluOpType
ACTF = mybir.ActivationFunctionType
U16 = mybir.dt.uint16

S = 512
D = 256
N_CORES = 8
SK_LOC = S // N_CORES  # 64 keys per core
GRP = 8                # keys per batch for DVE reduces / col math
NGRP = SK_LOC // GRP   # 16 groups
HALF_G = NGRP // 2     # groups per half (A2A overlap split)
C_SHIFT = 14.0
MM_DT = F16

_CACHE = {}


def _build():
    nc = bacc.Bacc(
        "TRN2",
        target_bir_lowering=False,
        debug=False,
        num_devices=N_CORES,
    )

    qT = nc.dram_tensor("qT", [D, S], F16, kind="ExternalInput").ap()
    wmod_ext = nc.dram_tensor("wmod_ext", [SK_LOC * 128, 2 * D], F16, kind="ExternalInput").ap()
    vT = nc.dram_tensor("vT", [128, SK_LOC * 2], F32, kind="ExternalInput").ap()
    blc = nc.dram_tensor("blc", [128, 2], F32, kind="ExternalInput").ap()
    ident = nc.dram_tensor("ident", [128, 128], BF16, kind="ExternalInput").ap()
    out_ext = nc.dram_tensor("out", [4 * 128, S], BF16, kind="ExternalOutput").ap()
    dbg = {}
    if os.environ.get("KV2_DEBUG"):
        dbg["e0"] = nc.dram_tensor("dbg_e0", [128, 2 * S], F32, kind="ExternalOutput").ap()
        dbg["xb0"] = nc.dram_tensor("dbg_xb0", [128, 2 * S], F32, kind="ExternalOutput").ap()
        dbg["xe0"] = nc.dram_tensor("dbg_xe0", [128, 2 * S], F32, kind="ExternalOutput").ap()
        dbg["ssum0"] = nc.dram_tensor("dbg_ssum0", [128, GRP * 2], F32, kind="ExternalOutput").ap()
        dbg["maxe0"] = nc.dram_tensor("dbg_maxe0", [128, GRP * 2], F32, kind="ExternalOutput").ap()
        dbg["coeff0"] = nc.dram_tensor("dbg_coeff0", [128, GRP * 2], F32, kind="ExternalOutput").ap()

    with tile.TileContext(nc) as tc:
        _emit(nc, tc, locals())
    nc.compile()
    return nc


def _emit(nc, tc, io):
    qT, wmod_ext, vT = io["qT"], io["wmod_ext"], io["vT"]
    blc = io["blc"]
    ident, out_ext = io["ident"], io["out_ext"]
    dbg = io["dbg"]

    import contextlib

    ctx = contextlib.ExitStack()
    with ctx:
        const = ctx.enter_context(tc.tile_pool(name="const", bufs=1))
        wmod_p = ctx.enter_context(tc.tile_pool(name="wmod", bufs=6))
        x_ps = ctx.enter_context(tc.tile_pool(name="x_ps", bufs=3, space="PSUM"))
        vs_ps = ctx.enter_context(tc.tile_pool(name="vs_ps", bufs=1, space="PSUM"))
        egrp_p = ctx.enter_context(tc.tile_pool(name="egrp", bufs=2))
        xbgrp_p = ctx.enter_context(tc.tile_pool(name="xbgrp", bufs=2))
        xegrp_p = ctx.enter_context(tc.tile_pool(name="xegrp", bufs=2))
        t1_p = ctx.enter_context(tc.tile_pool(name="t1", bufs=3))
        sa_p = ctx.enter_context(tc.tile_pool(name="sa", bufs=2))
        t2_p = ctx.enter_context(tc.tile_pool(name="t2", bufs=2))
        col_p = ctx.enter_context(tc.tile_pool(name="col", bufs=8))
        diag_p = ctx.enter_context(tc.tile_pool(name="diag", bufs=2))
        fpool = ctx.enter_context(tc.tile_pool(name="fpool", bufs=2))
        cmb = ctx.enter_context(tc.tile_pool(name="cmb", bufs=1))
        dram = ctx.enter_context(tc.tile_pool(name="dram", bufs=1, space="DRAM"))

        # ---- constants into SBUF -------------------------------------------
        qT_sb = const.tile([128, 2, S], F16)
        for m in range(2):
            nc.sync.dma_start(out=qT_sb[:, m, :], in_=qT[128 * m : 128 * (m + 1), :])
        PF = 6
        wmod_sbs = {}

        def fetch_wmod(k):
            t = wmod_p.tile([128, 2, D], F16, tag="wmod", name="wmod")
            nc.sync.dma_start(out=t, in_=wmod_ext[128 * k : 128 * (k + 1), :])
            wmod_sbs[k] = t

        for _k in range(4):
            fetch_wmod(_k)
        blc_sb = const.tile([128, 2], F32)
        nc.sync.dma_start(out=blc_sb, in_=blc)
        vT_sb = const.tile([128, SK_LOC, 2], F32)
        nc.sync.dma_start(out=vT_sb, in_=vT)
        ident_sb = const.tile([128, 128], BF16)
        nc.sync.dma_start(out=ident_sb, in_=ident)
        identg_sb = const.tile([128, 2 * GRP, 128], BF16)
        for _ii in range(2 * GRP):
            nc.vector.tensor_copy(out=identg_sb[:, _ii, :], in_=ident_sb)

        negc_sb = const.tile([128, 1], F32)
        nc.vector.memset(negc_sb, -C_SHIFT)

        for k in range(4, PF):
            fetch_wmod(k)

        # ---- PE warm-up: long continuous stretch to reach full p-state -----
        warm = const.tile([128, S], F16)
        nc.vector.memset(warm, 0.0)
        for _ in range(8):
            wm_ps = x_ps.tile([128, 2, S], F32, tag="x")
            nc.tensor.matmul(wm_ps[:, 0, 0:64], lhsT=warm[:, 0:128], rhs=warm[:, 0:64], start=True, stop=True)

        # ---- main loop ------------------------------------------------------
        vsum_ps = vs_ps.tile([128, 2, S], F32)
        e_grps = {}
        xb_grps = {}
        xe_grps = {}
        t2_grps = {}
        coeffs = {}
        diags = {}

        def emit_diag_apply_key(g, jj):
            """PE: apply group g's diag(coeff) matmuls for key jj into vsum."""
            for m in range(2):
                first = (g % HALF_G == 0) and jj == 0
                last = (g % HALF_G == HALF_G - 1) and jj == GRP - 1
                nc.tensor.matmul(
                    vsum_ps[:, m, :],
                    lhsT=diags[g][:, jj, m, :],
                    rhs=xe_grps[g][:, jj, m, :],
                    start=first,
                    stop=last,
                )

        def emit_diag_apply(g):
            for jj in range(GRP):
                emit_diag_apply_key(g, jj)

        def emit_drain_and_a2a(half):
            """Drain vsum (raw, bf16) straight to DRAM; host applies (I+Wvo).
            Half A's copies run on DVE (ACT is busier mid-kernel); the tail
            drain stays on ACT, which is idle by then."""
            vs_sb = fpool.tile([128, 2, S], BF16, tag="vs")
            for m in range(2):
                if half == 0:
                    nc.vector.tensor_copy(out=vs_sb[:, m, :], in_=vsum_ps[:, m, :])
                else:
                    nc.scalar.copy(out=vs_sb[:, m, :], in_=vsum_ps[:, m, :])
                nc.sync.dma_start(
                    out=out_ext[(half * 2 + m) * 128 : (half * 2 + m + 1) * 128, :],
                    in_=vs_sb[:, m, :],
                )

        for k in range(SK_LOC):
            g, j = divmod(k, GRP)

            # prefetch wmod for key k+PF
            if k + PF < SK_LOC:
                fetch_wmod(k + PF)

            # group-boundary: allocate fresh group tiles
            if j == 0:
                e_grps[g] = egrp_p.tile([128, GRP, 2, S], BF16, tag="e", name="e_grp")
                xb_grps[g] = xbgrp_p.tile([128, GRP, 2, S], BF16, tag="xb", name="xb_grp")
                xe_grps[g] = xegrp_p.tile([128, GRP, 2, S], BF16, tag="xe", name="xe_grp")
                t2_grps[g] = t2_p.tile([128, GRP, 2, 128], BF16, tag="t2", name="t2_grp")

            # PE: previous group's diag matmuls, one key per iteration
            if g >= 1:
                emit_diag_apply_key(g - 1, j)

            # PE: logits for key k
            wm = wmod_sbs.pop(k)
            x_psum = x_ps.tile([128, 2, S], F32, tag="x")
            for m in range(2):
                for kk in range(2):
                    nc.tensor.matmul(
                        x_psum[:, m, :],
                        lhsT=wm[:, kk, 128 * m : 128 * (m + 1)],
                        rhs=qT_sb[:, kk, :],
                        start=(kk == 0),
                        stop=(kk == 1),
                    )

            # ACT: e' and xb streams
            nc.scalar.activation(
                e_grps[g][:, j, :, :], x_psum, ACTF.Exp, bias=negc_sb[:], scale=1.0
            )
            for m in range(2):
                nc.scalar.activation(
                    xb_grps[g][:, j, m, :], x_psum[:, m, :], ACTF.Identity,
                    bias=blc_sb[:, m : m + 1], scale=1.0,
                )

            # DVE: maxe tree L1, L2 batched per 4 keys (2 for the split last group)
            tw = 2 if g == NGRP - 1 else 4
            if j % tw == tw - 1:
                t1 = t1_p.tile([128, 4, 2, S // 2], BF16, tag="t1", name="t1")
                nc.vector.tensor_tensor(
                    out=t1[:, 0:tw],
                    in0=e_grps[g][:, j - tw + 1 : j + 1, :, 0 : S // 2],
                    in1=e_grps[g][:, j - tw + 1 : j + 1, :, S // 2 : S],
                    op=ALU.max,
                )
                nc.vector.tensor_tensor(
                    out=t2_grps[g][:, j - tw + 1 : j + 1, :, :],
                    in0=t1[:, 0:tw, :, 0 : S // 4],
                    in1=t1[:, 0:tw, :, S // 4 : S // 2],
                    op=ALU.max,
                )

            # DVE: xexp mult (whole group; last group in 2-key chunks for tail)
            last_g = g == NGRP - 1
            if (j == GRP - 1 and not last_g) or (last_g and j % 2 == 1):
                j0, j1 = (0, GRP) if not last_g else (j - 1, j + 1)
                nc.vector.tensor_tensor(
                    out=xe_grps[g][:, j0:j1],
                    in0=xb_grps[g][:, j0:j1],
                    in1=e_grps[g][:, j0:j1],
                    op=ALU.mult,
                )

            if dbg and k == 1:
                for nm, t in [("e0", e_grps[0][:, 0, :, :]), ("xb0", xb_grps[0][:, 0, :, :]), ("xe0", xe_grps[0][:, 0, :, :])]:
                    tap = fpool.tile([128, 2, S], F32, tag="tap", name="tap")
                    nc.vector.tensor_copy(out=tap, in_=t)
                    nc.sync.dma_start(out=dbg[nm], in_=tap)

            if j == GRP - 1 and g == HALF_G:
                emit_drain_and_a2a(0)  # group HALF_G-1's diags all applied now
            # group end: reduces + col math + diag gen (DVE); last group split
            if (j == GRP - 1 and not last_g) or (last_g and j % 2 == 1):
                if last_g and g not in diags:
                    diags[g] = diag_p.tile([128, GRP, 2, 128], BF16, tag="diag", name="diag_grp")
                nj = j1 - j0
                sa = sa_p.tile([128, GRP, 2, S], BF16, tag="sa", name="sa")
                nc.vector.tensor_scalar(
                    out=sa[:, j0:j1].bitcast(U16), in0=xe_grps[g][:, j0:j1].bitcast(U16),
                    scalar1=0x7FFF, scalar2=None, op0=ALU.bitwise_and,
                )
                st = sa_p.tile([128, GRP, 2, S // 2], F16, tag="st", name="st")
                nc.vector.tensor_tensor(
                    out=st[:, j0:j1],
                    in0=sa[:, j0:j1, :, 0 : S // 2], in1=sa[:, j0:j1, :, S // 2 :],
                    op=ALU.add,
                )
                sb_ = st[:, j0:j1]
                for lvl in range(2):
                    half_w = S >> (lvl + 2)
                    nxt = st[:, j0:j1, :, 0:half_w]
                    nc.vector.tensor_tensor(
                        out=nxt, in0=sb_[:, :, :, 0:half_w], in1=sb_[:, :, :, half_w:],
                        op=ALU.add,
                    )
                    sb_ = nxt
                ssum = col_p.tile([128, GRP, 2], F32, tag="ssum", name="ssum")
                nc.vector.tensor_reduce(
                    out=ssum[:, j0:j1], in_=sb_, axis=AX.X, op=ALU.add,
                )
                maxe = col_p.tile([128, GRP, 2], F32, tag="maxe", name="maxe")
                nc.vector.tensor_reduce(
                    out=maxe[:, j0:j1], in_=t2_grps[g][:, j0:j1], axis=AX.X, op=ALU.max
                )
                den = col_p.tile([128, GRP, 2], F32, tag="den", name="den")
                nc.vector.tensor_tensor(out=den[:, j0:j1], in0=ssum[:, j0:j1], in1=maxe[:, j0:j1], op=ALU.add)
                rec = col_p.tile([128, GRP, 2], F32, tag="rec", name="rec")
                nc.vector.reciprocal(out=rec[:, j0:j1], in_=den[:, j0:j1])
                coeff = col_p.tile([128, GRP, 2], F32, tag="coeff", name="coeff")
                nc.vector.tensor_tensor(
                    out=coeff[:, j0:j1], in0=rec[:, j0:j1],
                    in1=vT_sb[:, g * GRP + j0 : g * GRP + j1, :], op=ALU.mult,
                )
                if not last_g:
                    diags[g] = diag_p.tile([128, GRP, 2, 128], BF16, tag="diag", name="diag_grp")
                dgrp = diags[g]
                bc = coeff[:, j0:j1, :, None].broadcast_to((128, nj, 2, 128))
                nc.vector.tensor_tensor(
                    out=dgrp[:, j0:j1],
                    in0=identg_sb[:, 0 : 2 * nj, :].rearrange("p (j m) f -> p j m f", j=nj, m=2),
                    in1=bc, op=ALU.mult,
                )
                if dbg and g == 0:
                    for nm, t in [("ssum0", ssum), ("maxe0", maxe), ("coeff0", coeff)]:
                        nc.sync.dma_start(out=dbg[nm], in_=t)
                if last_g:
                    for jj in range(j0, j1):
                        emit_diag_apply_key(g, jj)

        # tail: second half drain (last group's diags applied in-loop)
        emit_drain_and_a2a(1)




def get_nc():
    if "nc" not in _CACHE:
        _CACHE["nc"] = _build()
    return _CACHE["nc"]


def make_in_maps(inputs):
    """Host-side prep: residual linears, per-key weight folding, Sk shard."""
    f32 = np.float32
    f16 = np.float16
    q = inputs["query_tokens"][0].astype(f32)
    k = inputs["key_tokens"][0].astype(f32)
    v = inputs["value_tokens"][0].astype(f32)
    qq = q + q @ inputs["Wq"].astype(f32) + inputs["bq"].astype(f32)  # [S,D]
    kk = k + k @ inputs["Wk"].astype(f32) + inputs["bk"].astype(f32)
    vv = v + v @ inputs["Wva"].astype(f32) + inputs["bva"].astype(f32)
    eye = np.eye(D, dtype=f32)
    Wl1 = (eye + inputs["Wl"]).astype(f32)          # [din, dout]
    ident = np.eye(128, dtype=f32).astype(ml_dtypes.bfloat16)

    # qT [D, S]: row din = chunk*128 + p, matching the kernel's block DMA
    qT = np.ascontiguousarray(qq.T).astype(f16)

    # wmod_all[key, p, kc, d] = Wl1[kc*128+p, d] * kk[key, kc*128+p]
    W2 = Wl1.reshape(2, 128, D)                     # [kc, p, d]
    kk2 = kk.reshape(S, 2, 128)                     # [key, kc, p]
    wmod_all = (kk2[:, :, :, None] * W2[None]).transpose(0, 2, 1, 3)  # [key,p,kc,d]
    wmod_all = np.ascontiguousarray(wmod_all.reshape(S * 128, 2 * D)).astype(f16)

    # vT [128, key, m]: vT[p, key, m] = vv[key, m*128+p]
    vvr = vv.reshape(S, 2, 128)                     # [key, m, p]
    base = {
        "qT": qT,
        "blc": np.ascontiguousarray(inputs["bl"].reshape(2, 128).T, dtype=f32),
        "ident": ident,
    }
    in_maps = []
    for c in range(N_CORES):
        m = dict(base)
        sl = slice(c * SK_LOC, (c + 1) * SK_LOC)
        m["wmod_ext"] = np.ascontiguousarray(wmod_all[c * SK_LOC * 128 : (c + 1) * SK_LOC * 128])
        vt = vvr[sl].transpose(2, 0, 1).reshape(128, SK_LOC * 2)  # [p, key*m]
        m["vT"] = np.ascontiguousarray(vt).astype(f32)
        in_maps.append(m)
    return in_maps


def kernel(**inputs):
    nc = get_nc()
    in_maps = make_in_maps(inputs)
    res = run_bass_kernel_spmd(nc, in_maps, core_ids=list(range(N_CORES)))
    parts = np.stack([res.results[c]["out"].astype(np.float32) for c in range(N_CORES)])
    vsum = parts.reshape(N_CORES * 2, 2, 128, S).sum(axis=0)     # [m, p, sq]
    vsum = vsum.reshape(D, S).T                                   # [sq, dout]
    eye = np.eye(D, dtype=np.float32)
    out = vsum @ (eye + inputs["Wvo"].astype(np.float32)) + inputs["bvo"].astype(np.float32)
    return out.reshape(1, S, D).astype(np.float32)
